# revision 2
# baseline (speedup 1.0000x reference)
"""Trainium2 Bass kernel for nn_DeepManualLSTM (3-layer LSTM, B=1024, T=48, IN=64, H=512).

Data-parallel over batch (128 rows/core x 8 cores); weights SBUF-resident.

fp8-e4m3 DoubleRow matmuls (0.5 cycles/row) for the h-contractions on
timesteps 0..T-9; the last SUF=8 steps run in bf16.  LSTM forget gates
(~0.5 at these weight scales) exponentially attenuate early-step
quantization error, so the fp8 prefix contributes little to the final
error: measured rel err 9.5e-3 (all-bf16 baseline 7.2e-3, tolerance 2e-2).

All weights host-scaled x16 (exact in bf16; lifts fp8 out of subnormals);
gate ACTs apply scale=1/16 on the PSUM read.  Gate columns host-reordered
to [f,i,o,c]: one sigmoid ACT covers banks 0-2 (1536 contiguous columns),
one tanh covers bank 3.  PSUM per cell: one 3-bank fio tile + one 1-bank
c tile, bufs=2 => all 8 banks.

Orientation: transposed activations (feature-major) stationary, weights
moving, N=512 per PSUM bank.  DoubleRow operands are 3D APs [128, 2, free]
(virtual k row = plane*128 + partition).  Recurrent-part (Wh) matmuls are
emitted BEFORE the input-part (Wx/x) ones - the input h comes from the
immediately preceding cell, so the PE streams Wh work while it lands; the
c-gate bank finishes first so its tanh starts under the sigmoid banks'
last matmuls.  h re-enters the matmuls via 4 bf16 PE transposes into the
consumed fio PSUM + one converting DVE copy (bf16->fp8).  The 48-step
recurrence is fully unrolled in wavefront order (cells (s,0), (s-1,1),
(s-2,2)).  Final [B,H]@[H,1] projection runs on the host.
"""
import sys
import os

for _p in ("/opt/trn_rl_repo", "/root/.axon_site/_ro/trn_rl_repo"):
    if os.path.isdir(_p) and _p not in sys.path:
        sys.path.insert(0, _p)

import numpy as np
import ml_dtypes

import concourse.bass as bass
import concourse.tile as tile
from concourse import bacc, mybir
from concourse import bass_utils
from concourse.bass import ds, ts
from concourse.masks import make_identity

P = 128          # batch rows per core / SBUF partitions
T = 48           # sequence length
IN = 64          # input features
H = 512          # hidden size
L = 3            # layers
G4 = 4 * H       # gate width (2048)
NB = 4           # PSUM banks per gate row (G4 / 512)
KH = H // P      # k-chunks of the hidden contraction (4)
NCORES = 8
SUF = 8          # trailing timesteps computed in bf16
WS = 16.0        # weight pre-scale (gates ACT with scale=1/WS)

F32 = mybir.dt.float32
BF16 = mybir.dt.bfloat16
F8 = mybir.dt.float8e4
U16 = mybir.dt.uint16
AF = mybir.ActivationFunctionType
DR = mybir.MatmulPerfMode.DoubleRow


def _to_bf16(a: np.ndarray) -> np.ndarray:
    return np.ascontiguousarray(a, dtype=np.float32).astype(ml_dtypes.bfloat16)


def _to_f8(a: np.ndarray) -> np.ndarray:
    return np.ascontiguousarray(a, dtype=np.float32).astype(ml_dtypes.float8_e4m3)


def _build(include_bias: bool, reps: int = 1, for_sim: bool = False) -> bass.Bass:
    if for_sim:
        nc = bacc.Bacc(None, target_bir_lowering=False, debug=True)
    else:
        nc = bacc.Bacc()

    # x pre-transposed on the host: [128, (T//2)*128] bf16, even t in
    # partitions 0:64, odd t in 64:128, column block t//2 holds x_t^T.
    xT_d = nc.dram_tensor("xT", [P, (T // 2) * P], BF16, kind="ExternalInput")
    wx0_d = nc.dram_tensor("wx0", [IN, G4], BF16, kind="ExternalInput")
    wb_d = {  # bf16 weight set (x16)
        nm: nc.dram_tensor(nm, [H, G4], BF16, kind="ExternalInput")
        for nm in ("wh0", "wx1", "wh1", "wx2", "wh2")
    }
    w8_d = {  # fp8 weight set (x16)
        nm + "_8": nc.dram_tensor(nm + "_8", [H, G4], F8, kind="ExternalInput")
        for nm in ("wh0", "wx1", "wh1", "wx2", "wh2")
    }
    b_d = (
        [nc.dram_tensor(f"b{l}", [1, G4], BF16, kind="ExternalInput") for l in range(L)]
        if include_bias
        else None
    )
    # final h of the top layer back to the host (transposed bf16 layout)
    out_d = nc.dram_tensor("hout", [P, H], BF16, kind="ExternalOutput")

    with tile.TileContext(nc) as tc:
        with (
            tc.tile_pool(name="wpool", bufs=1) as wp,
            tc.tile_pool(name="state", bufs=1) as st,
            tc.tile_pool(name="work", bufs=3) as wk,
            tc.tile_pool(name="psg", bufs=2, space="PSUM") as psg,
        ):
            # ---- persistent tiles -------------------------------------------------
            identf = wp.tile([P, P], F32)
            make_identity(nc, identf)
            ident = wp.tile([P, P], BF16)
            nc.scalar.copy(ident[:], identf[:])

            xT_t = wp.tile([P, (T // 2) * P], BF16)
            nc.sync.dma_start(xT_t[:], xT_d[:])

            # Wx0 duplicated into both partition halves so odd-t x tiles
            # (living at base partition 64) find it on matching partitions.
            wx0_t = wp.tile([P, G4], BF16)
            nc.sync.dma_start(wx0_t[:IN, :], wx0_d[:])
            nc.sync.dma_start(wx0_t[IN:, :], wx0_d[:])
            # [H, G4] weights as [128, KH, G4]: partition = k % 128, k-chunk = k // 128
            big_w = {}
            for name, d in wb_d.items():
                w_t = wp.tile([P, KH, G4], BF16, name=f"{name}_t")
                nc.sync.dma_start(w_t[:], d.rearrange("(ko ki) n -> ki ko n", ki=P))
                big_w[name] = w_t
            w8 = {}
            for name, d in w8_d.items():
                w_t = wp.tile([P, KH, G4], F8, name=f"{name}_t")
                nc.sync.dma_start(w_t[:], d.rearrange("(ko ki) n -> ki ko n", ki=P))
                w8[name[:-2]] = w_t

            if include_bias:
                ones_f = wp.tile([1, P], F32)
                nc.vector.memset(ones_f[:], 1.0)
                ones_t = wp.tile([1, P], BF16)
                nc.scalar.copy(ones_t[:], ones_f[:])
                b_t = []
                for l in range(L):
                    bt = wp.tile([1, G4], BF16, name=f"b{l}_t")
                    nc.sync.dma_start(bt[:], b_d[l][:])
                    b_t.append(bt)

            # states: h transposed (feature-major) fp8 + bf16, C batch-major bf16.
            zbuf = wk.tile([P, H], F32, tag="zb")
            nc.vector.memset(zbuf[:], 0.0)
            hT8 = []   # [128, KH, 128] fp8, plane j = feature chunk j
            hTb = []   # [128, 512] bf16
            Cs = []
            for l in range(L):
                t8 = st.tile([P, KH, P], F8, name=f"hT8_{l}")
                nc.vector.memset(t8[:], 0.0)
                hT8.append(t8)
                tb = st.tile([P, H], BF16, name=f"hTb{l}")
                nc.scalar.copy(tb[:], zbuf[:])
                hTb.append(tb)
                c_t = st.tile([P, H], BF16, name=f"C{l}")
                nc.scalar.copy(c_t[:], zbuf[:])
                Cs.append(c_t)

            pending_finish = [None]

            def flush_pending():
                if pending_finish[0] is not None:
                    pending_finish[0][1]()
                    pending_finish[0] = None

            def cell(t: int, l: int, fp8: bool):
                """One LSTM cell update.

                Gate bank layout (host-reordered weight columns):
                banks 0-2 = f,i,o (one 1536-wide sigmoid ACT), bank 3 = c.
                The previous cell's transposes + hT copy ("finish") are
                deferred into this cell's emission to overlap with matmuls.
                """
                if pending_finish[0] is not None and pending_finish[0][0] == l:
                    flush_pending()

                fio = psg.tile([P, 3 * 512], F32, name="fio", tag="fio")
                gc = psg.tile([P, 512], F32, name="gc", tag="gc")

                def bank(n):
                    return fio[:, ts(n, 512)] if n < 3 else gc[:]

                started = [include_bias] * NB
                if include_bias:
                    for n in range(NB):
                        nc.tensor.matmul(
                            bank(n), ones_t[:], b_t[l][:, ts(n, 512)],
                            start=True, stop=False, skip_group_check=True,
                        )

                def mm(n, lhsT, rhs, last, perf_mode=None):
                    nc.tensor.matmul(
                        bank(n), lhsT, rhs,
                        start=not started[n], stop=last,
                        perf_mode=perf_mode, skip_group_check=True,
                    )
                    started[n] = True

                # Recurrent part (Wh) FIRST: its input h_l[t-1] landed three
                # cells ago, while the input part's h_{l-1}[t] comes from the
                # immediately preceding cell - so the PE streams Wh work while
                # that fresh input is still being produced.  The c-gate bank
                # (3) is finished first in the last dchunk so its tanh ACT can
                # start while the sigmoid banks' last matmuls stream.
                if fp8:
                    for c in range(KH // 2):
                        lhsT = hT8[l][:, 2 * c:2 * c + 2, :]
                        rhs = w8[f"wh{l}"]
                        for n in range(NB):
                            mm(n, lhsT, rhs[:, 2 * c:2 * c + 2, ts(n, 512)],
                               False, perf_mode=DR)
                    if l == 0:
                        r0 = 0 if t % 2 == 0 else 64
                        for n in (3, 0, 1, 2):
                            mm(n, xT_t[r0:r0 + IN, ts(t // 2, P)],
                               wx0_t[r0:r0 + IN, ts(n, 512)], True)
                    else:
                        for c in range(KH // 2):
                            lhsT = hT8[l - 1][:, 2 * c:2 * c + 2, :]
                            rhs = w8[f"wx{l}"]
                            last = c == KH // 2 - 1
                            for n in ((3, 0, 1, 2) if last else range(NB)):
                                mm(n, lhsT, rhs[:, 2 * c:2 * c + 2, ts(n, 512)],
                                   last, perf_mode=DR)
                else:
                    for k in range(KH):
                        lhsT = hTb[l][:, ts(k, P)]
                        rhs = big_w[f"wh{l}"]
                        for n in range(NB):
                            mm(n, lhsT, rhs[:, k, ts(n, 512)], False)
                    if l == 0:
                        r0 = 0 if t % 2 == 0 else 64
                        for n in (3, 0, 1, 2):
                            mm(n, xT_t[r0:r0 + IN, ts(t // 2, P)],
                               wx0_t[r0:r0 + IN, ts(n, 512)], True)
                    else:
                        for k in range(KH):
                            lhsT = hTb[l - 1][:, ts(k, P)]
                            rhs = big_w[f"wx{l}"]
                            last = k == KH - 1
                            for n in ((3, 0, 1, 2) if last else range(NB)):
                                mm(n, lhsT, rhs[:, k, ts(n, 512)], last)

                # Flush the previous cell's finish now - after this cell's
                # matmuls (so every emitted reader of the previous hT version
                # precedes the new write) but BEFORE this cell's gate-tail DVE
                # ops, so the hT copy doesn't queue behind them in DVE's FIFO.
                flush_pending()

                fio_s = wk.tile([P, 3 * H], BF16, tag="fio_s")
                c_s = wk.tile([P, H], BF16, tag="c_s")
                tanC = wk.tile([P, H], BF16, tag="tanC")
                fC = wk.tile([P, H], BF16, tag="fC")
                ic = wk.tile([P, H], BF16, tag="ic")

                f_s = fio_s[:, 0:H]
                i_s = fio_s[:, H:2 * H]
                o_s = fio_s[:, 2 * H:3 * H]
                # tanh for c first (its bank finished first), then one sigmoid
                # ACT for f,i,o; scale undoes WS
                nc.scalar.activation(c_s[:], gc[:], AF.Tanh, scale=1.0 / WS)
                nc.scalar.activation(fio_s[:], fio[:], AF.Sigmoid, scale=1.0 / WS)
                nc.vector.tensor_mul(fC[:], f_s, Cs[l][:])
                nc.vector.tensor_mul(ic[:], i_s, c_s[:])
                nc.vector.tensor_add(Cs[l][:], fC[:], ic[:])
                nc.scalar.activation(tanC[:], Cs[l][:], AF.Tanh)

                # h -> transposed state via 4 bf16 PE transposes into the
                # consumed fio PSUM (fp8 transpose needs stride-2 output APs,
                # so transpose in bf16 and convert in the DVE copy instead).
                h_b = wk.tile([P, H], BF16, tag="h_b")
                nc.vector.tensor_mul(h_b[:], o_s, tanC[:])
                dst = fio[:].bitcast(BF16)  # [P, 3072] bf16 view

                if fp8:
                    def finish(l=l, dst=dst, h_b=h_b):
                        for j in range(KH):
                            nc.tensor.transpose(
                                dst[:, ts(j, P)], h_b[:, ts(j, P)], ident[:]
                            )
                        nc.vector.tensor_copy(hT8[l][:], dst[:, 0:H])
                else:
                    def finish(l=l, dst=dst, h_b=h_b):
                        for j in range(KH):
                            nc.tensor.transpose(
                                dst[:, ts(j, P)], h_b[:, ts(j, P)], ident[:]
                            )
                        nc.vector.tensor_copy(hTb[l][:], dst[:, 0:H])

                pending_finish[0] = (l, finish)

            def whole_pass():
                # wavefront order: cells (s,0), (s-1,1), (s-2,2)
                for s in range(T + L - 1):
                    for l in range(L):
                        t = s - l
                        if 0 <= t < T:
                            if t == T - SUF:
                                # boundary: seed the bf16 state from fp8
                                nc.vector.tensor_copy(hTb[l][:], hT8[l][:])
                            cell(t, l, fp8=(t < T - SUF))
                flush_pending()

            if reps > 1:
                with tc.For_i(0, reps, 1):
                    whole_pass()
            else:
                whole_pass()

            flush_pending()

            # ---- ship final top-layer h back to the host --------------------------
            nc.sync.dma_start(out_d[:], hTb[L - 1][:])

    nc.finalize()
    return nc


_NC_CACHE: dict = {}
_LAST_RUN: dict = {}

# host-side gate reorder [f,i,c,o] -> [f,i,o,c]
_PERM = np.concatenate(
    [
        np.arange(0, H),          # f
        np.arange(H, 2 * H),      # i
        np.arange(3 * H, 4 * H),  # o
        np.arange(2 * H, 3 * H),  # c
    ]
)


def _pack_xT(x_shard: np.ndarray) -> np.ndarray:
    """[128, T, IN] -> [128, (T//2)*128] packed transposed layout (bf16)."""
    xt = np.zeros((P, (T // 2) * P), dtype=np.float32)
    for t in range(T):
        r0 = 0 if t % 2 == 0 else 64
        xt[r0: r0 + IN, (t // 2) * P: (t // 2 + 1) * P] = x_shard[:, t, :].T
    return _to_bf16(xt)


def _prep_weights(inputs):
    """Host-side: perm gate columns, x16 scale, bf16 + fp8 sets."""
    ws = {}
    for name in ("Wx0", "Wh0", "Wx1", "Wh1", "Wx2", "Wh2"):
        w = np.asarray(inputs[name], dtype=np.float32)[:, _PERM] * WS
        key = name.lower()
        ws[key] = _to_bf16(w)
        if name != "Wx0":
            ws[key + "_8"] = _to_f8(w)
    return ws


def kernel(**inputs) -> np.ndarray:
    x = np.ascontiguousarray(np.asarray(inputs["x"], dtype=np.float32))
    B = x.shape[0]
    assert B % NCORES == 0
    Bl = B // NCORES

    ws = _prep_weights(inputs)
    fc_w = np.asarray(inputs["fc_w"], dtype=np.float32)
    bs = [np.asarray(inputs[f"b{l}"], dtype=np.float32)[_PERM] for l in range(L)]
    fc_b = np.asarray(inputs["fc_b"], dtype=np.float32)
    include_bias = any(np.any(b != 0) for b in bs)

    key = include_bias
    if key not in _NC_CACHE:
        _NC_CACHE[key] = _build(include_bias)
    nc = _NC_CACHE[key]
    _LAST_RUN["include_bias"] = include_bias

    in_maps = []
    for c in range(NCORES):
        m = {"xT": _pack_xT(x[c * Bl: (c + 1) * Bl])}
        for nm in ("wx0", "wh0", "wx1", "wh1", "wx2", "wh2"):
            m[nm] = ws[nm]
            if nm != "wx0":
                m[nm + "_8"] = ws[nm + "_8"]
        if include_bias:
            for l in range(L):
                m[f"b{l}"] = _to_bf16(bs[l] * WS).reshape(1, G4)
        in_maps.append(m)

    res = bass_utils.run_bass_kernel_spmd(nc, in_maps, core_ids=list(range(NCORES)))
    _LAST_RUN["nc"] = nc
    _LAST_RUN["in_maps"] = in_maps
    outs = []
    for c in range(NCORES):
        hu = res.results[c]["hout"]  # [128, 512] bf16-as-u16 (or float)
        hu = np.asarray(hu)
        if hu.dtype == np.uint16:
            ht = (hu.astype(np.uint32) << 16).view(np.float32)
        else:
            ht = hu.astype(np.float32)
        # ht[p, 128*j + b] = h2[b, 128*j + p]
        h2 = ht.reshape(P, KH, P).transpose(2, 1, 0).reshape(P, H)
        outs.append(h2 @ fc_w)
    out = np.concatenate(outs, axis=0)
    return (out + fc_b.reshape(1, -1)).astype(np.float32)
